# revision 61
# baseline (speedup 1.0000x reference)
"""Trainium2 Bass kernel for causal multi-head attention.

Problem: x[64,256,512] f32, Wq/Wk/Wv[8,512,64], Wo[512,512]
  q,k,v = einsum('btc,hcd->bhtd'); scores = q k^T / sqrt(512) (causal);
  out = softmax(scores) v; y = concat-heads(out) @ Wo.

Strategy: data-parallel over batch across 8 NeuronCores (8 batches/core,
no collectives). Per core, for each batch b:
  - load host-pretransposed xT [512c, 256t]
  - QT/KT = W^T x^T via head-pair-packed matmuls -> [d, t] layout
    (two head-pairs share one PSUM bank -> one 512-wide copy out)
  - V' = x Wv -> [s, h, d] layout with a ones column appended per head
    (PV matmul then yields the softmax denominator for free)
  - per head: S^T[s,384] = [s0 x t(256) | s1 x t1(128)] in ONE psum tile;
    P = exp(scale*S) (single ACT op); multiplicative fp16 causal mask on
    the two diagonal blocks (gpsimd); fully-masked (t0,s1) never computed
  - PV: psum [t, 4head, 65] per (tb, head-quad); col 64 = denominator
  - normalize directly from PSUM: rec = 1/denoms (DVE), broadcast
    tensor_mul -> outn [t, tb, h, d] fp16
  - PE-transpose outn -> outT [hd, t]; y = outT^T Wo; DMA out.

Engine balance: exp is ACT-only; PSUM->SBUF copies are split between ACT
and DVE; causal mask multiplies run on gpsimd (SBUF-only engine).
Matmul operands are fp16 (1 cyc/row on PE vs 4 for fp32).
"""
import numpy as np

import concourse.bass as bass
import concourse.tile as tile
import concourse.mybir as mybir
from concourse import bacc
from concourse.bass_utils import run_bass_kernel_spmd

F32 = mybir.dt.float32
F16 = mybir.dt.float16
BF16 = mybir.dt.bfloat16
F8E4 = mybir.dt.float8e4
W8_SCALE = 8.0              # fp8 Wq/Wk pre-scale (avoids e4m3 subnormals)

N_CORES = 8
B, T, C = 64, 256, 512
H, DK = 8, 64
B_LOC = B // N_CORES        # 8 batches per core
N_HP = H // 2               # head pairs (2x64 packed on partitions)
N_CC = C // 128             # contraction chunks
SCALE = 1.0 / np.sqrt(np.float32(C))

_DT = {"fp32": F32, "fp16": F16, "bf16": BF16}


def build_nc(cfg, repeat=0, tune=None):
    """repeat=0: straight-line kernel. repeat=R>0: wrap the whole per-batch
    pipeline in a hardware For_i loop executed R times (for timing)."""
    tu = {"xT": 2, "qk": 2, "vp": 2, "pp": 4, "op": 2, "rp": 4, "yp": 2,
          "ps_proj": 2, "ps_s": 3, "ps_pv": 3,
          # engine split for PSUM->SBUF copies: s=ACT, v=DVE, g=gpsimd
          "eng_qk": "svsv", "eng_vv": "sv", "eng_outT": "vvvv",
          "eng_yt": "sv", "eng_mask": "gggggggg",
          # mask_mode "mul": multiplicative fp16 mask after exp (eng_mask)
          # mask_mode "pe": additive -big mask folded into the scores
          #                 matmul accumulation on the tensor engine
          # mask_mode "pre": PSUM diag regions pre-initialized with the
          #                  -big mask by a copy; scores accumulate on top
          "mask_mode": "pe", "eng_pre": "ssssssss",
          # ahead: how many heads the score matmuls run ahead of PV
          "ahead": 1,
          # sc_order 2: cluster the two ks0-stationary score matmuls
          "sc_order": 1,
          # ydge "s": dispatch y-store DMAs from the ACT DGE ring
          "ydge": "sync",
          # exp_pair: two heads' scores share one 2-bank PSUM tile and
          # a single exp instruction (halves ACT hops in the head loop)
          "exp_pair": 0,
          # norm "dve": broadcast tensor_mul out of PSUM (1 DVE op/quad)
          # norm "act": per-head ACT copy with per-partition 1/denom scale
          "norm": "dve",
          # imap 2: spread proj-group inserts to also cover late heads
          "imap": 2,
          # yfp16: store y in fp16 (host upcasts); halves y DMA bytes
          "yfp16": 0,
          # transp "pe": PE transpose + PSUM->SBUF copy; "dma": XBAR DMA
          "transp": "pe"}
    tu.update(DEFAULT_TUNE)
    tu.update(tune or {})
    if tu["exp_pair"]:
        # 2-bank score tiles: fit PSUM by shrinking the s/pv pools
        tu["ps_s"] = min(tu["ps_s"], 2)
        tu["ps_pv"] = min(tu["ps_pv"], 2)
    qk8 = cfg.get("qkproj", "") == "fp8"   # Q/K projections via fp8 DoubleRow
    v8 = cfg.get("vproj", "") == "fp8"     # V projection via fp8 DoubleRow
    proj_dt = _DT[cfg["proj"]]     # x / Wq / Wk / Wv operand dtype
    sc_dt = _DT[cfg["scores"]]     # QT / KT operand dtype
    pv_dt = _DT[cfg["pv"]]         # P / V' operand dtype
    op_dt = _DT[cfg["outproj"]]    # outT / Wo operand dtype

    nc = bacc.Bacc("TRN2", target_bir_lowering=False, debug=False)

    def cp(engine, out, in_):
        if engine == "s":
            nc.scalar.copy(out, in_)
        elif engine == "v":
            nc.vector.tensor_copy(out, in_)
        else:
            nc.gpsimd.tensor_copy(out, in_)

    qk_dt = F8E4 if qk8 else proj_dt
    v_dt = F8E4 if v8 else proj_dt
    xT_d = nc.dram_tensor("xT", [B_LOC, C, T], proj_dt, kind="ExternalInput").ap()
    if qk8 or v8:
        xT8_d = nc.dram_tensor("xT8", [B_LOC, C, T], F8E4, kind="ExternalInput").ap()
    wq_d = nc.dram_tensor("wq", [128, N_HP * N_CC * 128], qk_dt, kind="ExternalInput").ap()
    wk_d = nc.dram_tensor("wk", [128, N_HP * N_CC * 128], qk_dt, kind="ExternalInput").ap()
    wv_d = nc.dram_tensor("wv", [128, N_CC * C], v_dt, kind="ExternalInput").ap()
    wo_d = nc.dram_tensor("wo", [128, N_CC * C], op_dt, kind="ExternalInput").ap()
    mask_d = nc.dram_tensor("mask", [128, 384], pv_dt, kind="ExternalInput").ap()
    negm_d = nc.dram_tensor("negm", [128, 128], sc_dt, kind="ExternalInput").ap()
    ident_d = nc.dram_tensor("ident", [128, 128], op_dt, kind="ExternalInput").ap()
    identm_d = nc.dram_tensor("identm", [128, 128], sc_dt, kind="ExternalInput").ap()
    y_dt = F16 if tu["yfp16"] else F32
    y_d = nc.dram_tensor("y", [B_LOC, T, C], y_dt, kind="ExternalOutput").ap()

    with tile.TileContext(nc) as tc:
        import contextlib
        ctx = contextlib.ExitStack()
        with ctx:
            const = ctx.enter_context(tc.tile_pool(name="const", bufs=1))
            xT_p = ctx.enter_context(tc.tile_pool(name="xT", bufs=tu["xT"]))
            qk_p = ctx.enter_context(tc.tile_pool(name="qk", bufs=tu["qk"]))
            vp_p = ctx.enter_context(tc.tile_pool(name="vp", bufs=tu["vp"]))
            p_p = ctx.enter_context(tc.tile_pool(name="pp", bufs=tu["pp"]))
            o_p = ctx.enter_context(tc.tile_pool(name="op", bufs=tu["op"]))
            r_p = ctx.enter_context(tc.tile_pool(name="rp", bufs=tu["rp"]))
            y_p = ctx.enter_context(tc.tile_pool(name="yp", bufs=tu["yp"]))
            ps_proj = ctx.enter_context(tc.tile_pool(name="ps_proj", bufs=tu["ps_proj"], space="PSUM"))
            ps_s = ctx.enter_context(tc.tile_pool(name="ps_s", bufs=tu["ps_s"], space="PSUM"))
            ps_pv = ctx.enter_context(tc.tile_pool(name="ps_pv", bufs=tu["ps_pv"], space="PSUM"))

            # persistent constants / weights
            mask = const.tile([128, 384], pv_dt)
            nc.sync.dma_start(mask[:], mask_d[:])
            negm = const.tile([128, 128], sc_dt)
            nc.sync.dma_start(negm[:], negm_d[:])
            ident = const.tile([128, 128], op_dt)
            nc.sync.dma_start(ident[:], ident_d[:])
            identm = const.tile([128, 128], sc_dt)
            nc.sync.dma_start(identm[:], identm_d[:])
            wq = const.tile([128, N_HP, N_CC, 128], qk_dt)
            nc.sync.dma_start(wq[:], wq_d.rearrange("p (a b c) -> p a b c", a=N_HP, b=N_CC))
            wk = const.tile([128, N_HP, N_CC, 128], qk_dt)
            nc.sync.dma_start(wk[:], wk_d.rearrange("p (a b c) -> p a b c", a=N_HP, b=N_CC))
            wv = const.tile([128, N_CC, C], v_dt)
            nc.sync.dma_start(wv[:], wv_d.rearrange("p (a b) -> p a b", a=N_CC))
            wo = const.tile([128, N_CC, C], op_dt)
            nc.sync.dma_start(wo[:], wo_d.rearrange("p (a b) -> p a b", a=N_CC))

            def emit_load(st):
                # ---- load xT for this batch: [128, cc, 256] ----
                st["xT"] = xT = xT_p.tile([128, N_CC, T], proj_dt, tag="xT",
                                          name="xT_t")
                nc.sync.dma_start(
                    xT[:], xT_d[st["b"]].rearrange("(a p) t -> p a t", p=128))
                if qk8 or v8:
                    st["xT8"] = xT8 = xT_p.tile([128, N_CC, T], F8E4,
                                                tag="xT8", name="xT8_t")
                    nc.sync.dma_start(
                        xT8[:],
                        xT8_d[st["b"]].rearrange("(a p) t -> p a t", p=128))

            def emit_proj_group(st, gi):
                # ---- projections, one PSUM-bank group at a time ----
                # gi 0,1: Q head-pair-pairs; 2,3: K; 4,5: V s-chunks.
                # Two head-pairs share one PSUM bank -> one 512-wide copy.
                xT = st["xT"]
                if gi < 4:
                    w = (wq, wk)[gi // 2]
                    if gi == 0:
                        st["qt"] = qk_p.tile([128, N_HP, T], sc_dt, tag="qt",
                                             name="qt_t")
                    if gi == 2:
                        st["kt"] = qk_p.tile([128, N_HP, T], sc_dt, tag="kt",
                                             name="kt_t")
                    dst = st["qt"] if gi < 2 else st["kt"]
                    hpp = gi % 2
                    ps = ps_proj.tile([128, 512], F32, tag="ps_proj")
                    for hp2 in range(2):
                        hp = 2 * hpp + hp2
                        if qk8:
                            # fp8 DoubleRow: two K-chunks per matmul
                            xT8 = st["xT8"]
                            for j in range(N_CC // 2):
                                nc.tensor.matmul(
                                    ps[:, hp2 * T:(hp2 + 1) * T],
                                    w[:, hp, 2 * j:2 * j + 2, :],
                                    xT8[:, 2 * j:2 * j + 2, :],
                                    start=(j == 0), stop=(j == N_CC // 2 - 1),
                                    perf_mode=mybir.MatmulPerfMode.DoubleRow)
                        else:
                            for cc in range(N_CC):
                                nc.tensor.matmul(
                                    ps[:, hp2 * T:(hp2 + 1) * T],
                                    w[:, hp, cc, :], xT[:, cc, :],
                                    start=(cc == 0), stop=(cc == N_CC - 1))
                    cp(tu["eng_qk"][gi],
                       dst[:, 2 * hpp:2 * hpp + 2, :],
                       ps[:].rearrange("p (a t) -> p a t", a=2))
                else:
                    # V projection -> V' [128s, schunk, h, 65]
                    sc = gi - 4
                    if sc == 0:
                        st["vv"] = vp_p.tile([128, 2, H, 65], pv_dt, tag="vv",
                                             name="vv_t")
                    vv = st["vv"]
                    ps = ps_proj.tile([128, 512], F32, tag="ps_proj")
                    if v8:
                        xT8 = st["xT8"]
                        for j in range(N_CC // 2):
                            nc.tensor.matmul(
                                ps[:], xT8[:, 2 * j:2 * j + 2, bass.ts(sc, 128)],
                                wv[:, 2 * j:2 * j + 2, :],
                                start=(j == 0), stop=(j == N_CC // 2 - 1),
                                perf_mode=mybir.MatmulPerfMode.DoubleRow)
                    else:
                        for cc in range(N_CC):
                            nc.tensor.matmul(
                                ps[:], xT[:, cc, bass.ts(sc, 128)], wv[:, cc, :],
                                start=(cc == 0), stop=(cc == N_CC - 1))
                    cp(tu["eng_vv"][sc],
                       vv[:, sc, :, 0:DK],
                       ps[:].rearrange("p (h d) -> p h d", h=H))
                    if sc == 1:
                        # v8: wv is pre-scaled by W8_SCALE; an equally scaled
                        # denominator column cancels it in the normalize
                        nc.vector.memset(vv[:, :, :, DK:65],
                                         W8_SCALE if v8 else 1.0)

            def emit_scores(st, h):
                qt, kt = st["qt"], st["kt"]
                hp, lo = h // 2, (h % 2) * DK
                qs = qt[lo:lo + DK, hp, :]
                ks = kt[lo:lo + DK, hp, :]
                # S^T in one tile: [s0 x t(0:256) | s1 x t1(256:384)]
                ss = ps_s.tile([128, 384], F32, tag="s", name=f"s_{h}")
                if tu["mask_mode"] == "pre":
                    # pre-init both diag regions with -big causal mask in one
                    # strided copy; the diag score matmuls accumulate on top
                    dst = ss[:].rearrange("p (a c) -> p a c", c=128)[:, 0:3:2, :]
                    src = negm[:].rearrange("p (a c) -> p a c", a=1)
                    cp(tu["eng_pre"][h], dst, src.to_broadcast([128, 2, 128]))
                    nc.tensor.matmul(ss[:, 0:128], ks[:, 0:128], qs[:, 0:128],
                                     start=False, stop=True,
                                     skip_group_check=True)
                    nc.tensor.matmul(ss[:, 128:256], ks[:, 0:128],
                                     qs[:, 128:256], start=True, stop=True)
                    nc.tensor.matmul(ss[:, 256:384], ks[:, 128:256],
                                     qs[:, 128:256], start=False, stop=True,
                                     skip_group_check=True)
                elif tu["mask_mode"] == "pe" and tu.get("mask_adj"):
                    # same as "pe" but the two identm mask matmuls are
                    # emitted back-to-back (single stationary reload)
                    nc.tensor.matmul(ss[:, 0:128], ks[:, 0:128],
                                     qs[:, 0:128], start=True, stop=False)
                    nc.tensor.matmul(ss[:, 128:256], ks[:, 0:128],
                                     qs[:, 128:256], start=True, stop=True)
                    nc.tensor.matmul(ss[:, 256:384], ks[:, 128:256],
                                     qs[:, 128:256], start=True, stop=False)
                    nc.tensor.matmul(ss[:, 0:128], identm[:], negm[:],
                                     start=False, stop=True)
                    nc.tensor.matmul(ss[:, 256:384], identm[:], negm[:],
                                     start=False, stop=True)
                elif tu["mask_mode"] == "pe" and tu["sc_order"] == 2:
                    # same math; the two ks0-stationary matmuls adjacent
                    # (single stationary load), mask follows
                    nc.tensor.matmul(ss[:, 0:128], ks[:, 0:128],
                                     qs[:, 0:128], start=True, stop=False)
                    nc.tensor.matmul(ss[:, 128:256], ks[:, 0:128],
                                     qs[:, 128:256], start=True, stop=True)
                    nc.tensor.matmul(ss[:, 0:128], identm[:], negm[:],
                                     start=False, stop=True)
                    nc.tensor.matmul(ss[:, 256:384], ks[:, 128:256],
                                     qs[:, 128:256], start=True, stop=False)
                    nc.tensor.matmul(ss[:, 256:384], identm[:], negm[:],
                                     start=False, stop=True)
                elif tu["mask_mode"] == "pe":
                    # additive -big causal mask folded into the PSUM
                    # accumulation of the two diagonal blocks
                    nc.tensor.matmul(ss[:, 0:128], ks[:, 0:128],
                                     qs[:, 0:128], start=True, stop=False)
                    nc.tensor.matmul(ss[:, 0:128], identm[:], negm[:],
                                     start=False, stop=True)
                    nc.tensor.matmul(ss[:, 128:256], ks[:, 0:128],
                                     qs[:, 128:256], start=True, stop=True)
                    nc.tensor.matmul(ss[:, 256:384], ks[:, 128:256],
                                     qs[:, 128:256], start=True, stop=False)
                    nc.tensor.matmul(ss[:, 256:384], identm[:], negm[:],
                                     start=False, stop=True)
                else:
                    nc.tensor.matmul(ss[:, 0:T], ks[:, 0:128], qs[:],
                                     start=True, stop=True)
                    nc.tensor.matmul(ss[:, T:384], ks[:, 128:256],
                                     qs[:, 128:256], start=True, stop=True)
                # P = exp(scale * S); qk8 mode folds the 1/W8_SCALE^2 of the
                # pre-scaled fp8 Wq/Wk into the exp scale
                sscale = SCALE / (W8_SCALE * W8_SCALE) if qk8 else SCALE
                p = p_p.tile([128, 384], pv_dt, tag="p", name=f"p_{h}")
                nc.scalar.activation(p[:], ss[:],
                                     mybir.ActivationFunctionType.Exp,
                                     bias=0.0, scale=float(sscale))
                if tu["mask_mode"] == "mul":
                    me = tu["eng_mask"][h]
                    mop = (nc.gpsimd if me == "g" else nc.vector).tensor_mul
                    pm = p_p.tile([128, 384], pv_dt, tag="pm", name=f"pm_{h}")
                    mop(pm[:], p[:], mask[:])
                    p = pm
                st[f"p{h}"] = p

            def emit_scores_pair(st, pr):
                # scores for heads 2pr, 2pr+1 (same head-pair hp) in one
                # 2-bank PSUM tile; a single exp covers both heads
                qt, kt = st["qt"], st["kt"]
                ss = ps_s.tile([128, 1024], F32, tag="s", name=f"sp_{pr}")
                for j in range(2):
                    h = 2 * pr + j
                    lo = j * DK
                    qs = qt[lo:lo + DK, pr, :]
                    ks = kt[lo:lo + DK, pr, :]
                    o = 512 * j
                    nc.tensor.matmul(ss[:, o:o + 128], ks[:, 0:128],
                                     qs[:, 0:128], start=True, stop=False)
                    nc.tensor.matmul(ss[:, o:o + 128], identm[:], negm[:],
                                     start=False, stop=True)
                    nc.tensor.matmul(ss[:, o + 128:o + 256], ks[:, 0:128],
                                     qs[:, 128:256], start=True, stop=True)
                    nc.tensor.matmul(ss[:, o + 256:o + 384], ks[:, 128:256],
                                     qs[:, 128:256], start=True, stop=False)
                    nc.tensor.matmul(ss[:, o + 256:o + 384], identm[:],
                                     negm[:], start=False, stop=True)
                sscale = SCALE / (W8_SCALE * W8_SCALE) if qk8 else SCALE
                p = p_p.tile([128, 2, 384], pv_dt, tag="p", name=f"pq_{pr}")
                src = ss[:].rearrange("p (a c) -> p a c", c=512)[:, :, 0:384]
                nc.scalar.activation(p[:], src,
                                     mybir.ActivationFunctionType.Exp,
                                     bias=0.0, scale=float(sscale))
                st[f"p{2 * pr}"] = p[:, 0, :]
                st[f"p{2 * pr + 1}"] = p[:, 1, :]

            def emit_pv(st, h):
                vv, p = st["vv"], st.pop(f"p{h}")
                outn, pvt = st["outn"], st["pvt"]
                # PV into per-(tb, head-quad) psum [t, 4, 65]
                tb_jobs = ((0, ((p[:, 0:128], 0, True, True),)),
                           (1, ((p[:, 128:256], 0, True, False),
                                (p[:, 256:384], 1, False, True))))
                for tb, jobs in tb_jobs:
                    key, q4 = (tb, h // 4), h % 4
                    if q4 == 0:
                        pvt[key] = ps_pv.tile([128, 4, 65], F32, tag="pv",
                                              name=f"pv_{tb}_{h // 4}")
                    for (pslice, sc, st_, sp) in jobs:
                        nc.tensor.matmul(pvt[key][:, q4, :], pslice,
                                         vv[:, sc, h, :],
                                         start=st_, stop=sp)
                    if q4 == 3:
                        # normalize straight out of PSUM
                        tile_ = pvt.pop(key)
                        rec = r_p.tile([128, 4, 1], F32, tag="rec")
                        nc.vector.reciprocal(rec[:, :, 0], tile_[:, :, DK])
                        hq = 4 * (h // 4)
                        if tu["norm"] == "act":
                            for q in range(4):
                                nc.scalar.activation(
                                    outn[:, tb, hq + q, :],
                                    tile_[:, q, 0:DK],
                                    mybir.ActivationFunctionType.Copy,
                                    bias=0.0, scale=rec[:, q, :])
                        else:
                            nc.vector.tensor_mul(
                                outn[:, tb, hq:hq + 4, :],
                                tile_[:, :, 0:DK],
                                rec[:].to_broadcast([128, 4, DK]))

            def emit_transp(st, ccp):
                # ---- transpose outn [t,(h d)] -> outT [hd, tb*128] ----
                outn, outT = st["outn"], st["outT"]
                for tb in range(2):
                    if tu["transp"] == "dma":
                        for c2 in range(2):
                            cc = 2 * ccp + c2
                            nc.sync.dma_start_transpose(
                                outT[:, cc, bass.ts(tb, 128)],
                                outn[:, tb, 2 * cc:2 * cc + 2, :])
                        continue
                    tp = ps_s.tile([128, 256], op_dt, tag="s",
                                   name=f"tp_{tb}_{ccp}")
                    for c2 in range(2):
                        cc = 2 * ccp + c2
                        nc.tensor.transpose(
                            tp[:, bass.ts(c2, 128)],
                            outn[:, tb, 2 * cc:2 * cc + 2, :], ident[:])
                    cp(tu["eng_outT"][2 * tb + ccp],
                       outT[:, 2 * ccp:2 * ccp + 2, bass.ts(tb, 128)],
                       tp[:].rearrange("p (a t) -> p a t", a=2))

            def emit_heads(st, nxt):
                # head loop for batch b, with the next batch's projection
                # groups interleaved into the PE idle slots (the head loop
                # is ACT-paced: exp > per-head PE work).
                st["outn"] = o_p.tile([128, 2, H, DK], op_dt, tag="outn",
                                      name="outn_t")
                st["outT"] = o_p.tile([128, N_CC, T], op_dt, tag="outT",
                                      name="outT_t")
                st["pvt"] = {}
                if tu["exp_pair"]:
                    inserts = {0: (0, 1), 1: (2, 3), 2: (4,)}
                    emit_scores_pair(st, 0)
                    for pr in range(H // 2):
                        if pr + 1 < H // 2:
                            emit_scores_pair(st, pr + 1)
                        emit_pv(st, 2 * pr)
                        emit_pv(st, 2 * pr + 1)
                        if nxt is not None:
                            for gi in inserts.get(pr, ()):
                                emit_proj_group(nxt, gi)
                        if pr == 2:
                            emit_transp(st, 0)
                    return
                ahead = tu["ahead"]
                if tu["imap"] == 2:
                    imap = {0: 0, 1: 1, 2: 2, 3: 3, 6: 4}
                else:
                    imap = {0: 0, 1: 1, 2: 2, 3: 3, 4: 4}
                for h in range(ahead):
                    emit_scores(st, h)
                for h in range(H):
                    if h + ahead < H:
                        emit_scores(st, h + ahead)
                    emit_pv(st, h)
                    if nxt is not None and h in imap:
                        emit_proj_group(nxt, imap[h])
                    if h == 5 and ahead == 1:
                        emit_transp(st, 0)

            def emit_tail(st, nxt):
                # V s-chunk-1 projection of the next batch covers the
                # normalize latency of head-quad 1 before its transposes
                if tu["ahead"] != 1 and not tu["exp_pair"]:
                    emit_transp(st, 0)
                if nxt is not None:
                    emit_proj_group(nxt, 5)
                emit_transp(st, 1)
                # ---- output projection ----
                outT, b = st["outT"], st["b"]
                for tb in range(2):
                    ps = ps_proj.tile([128, 512], F32, tag="ps_proj")
                    for cc in range(N_CC):
                        nc.tensor.matmul(
                            ps[:], outT[:, cc, bass.ts(tb, 128)], wo[:, cc, :],
                            start=(cc == 0), stop=(cc == N_CC - 1))
                    yt = y_p.tile([128, C], y_dt, tag="yt")
                    cp(tu["eng_yt"][tb], yt[:], ps[:])
                    ydge = nc.scalar if tu["ydge"] == "s" else nc.sync
                    ydge.dma_start(y_d[b, bass.ts(tb, 128), :], yt[:])

            def batch_pipeline():
                # software pipeline across batches: batch b+1's projections
                # are interleaved into batch b's head loop and tail so PE
                # never waits on the exp/normalize chains.
                sts = [{"b": b} for b in range(B_LOC)]
                emit_load(sts[0])
                for gi in range(6):
                    emit_proj_group(sts[0], gi)
                for b in range(B_LOC):
                    nxt = sts[b + 1] if b + 1 < B_LOC else None
                    if nxt is not None:
                        emit_load(nxt)
                    emit_heads(sts[b], nxt)
                    emit_tail(sts[b], nxt)

            if repeat:
                with tc.For_i(0, repeat, 1):
                    batch_pipeline()
            else:
                batch_pipeline()

    nc.compile()
    return nc


def _prep_inputs(x, Wq, Wk, Wv, Wo, cfg):
    """Host-side reshapes/casts. Returns per-core input maps."""
    def np_dt(name):
        if name == "fp16":
            return np.float16
        if name == "bf16":
            import ml_dtypes
            return ml_dtypes.bfloat16
        return np.float32

    proj_np = np_dt(cfg["proj"])
    pv_np = np_dt(cfg["pv"])
    op_np = np_dt(cfg["outproj"])

    # weights: head-pair stationary blocks [hp, cc, 128c, 128d2] -> [128, hp*cc*128]
    qk_np = mybir.dt.np(F8E4) if cfg.get("qkproj", "") == "fp8" else np_dt(cfg["proj"])
    qk_scale = W8_SCALE if cfg.get("qkproj", "") == "fp8" else 1.0

    def pack_qk(w):
        w2 = np.ascontiguousarray(w.transpose(1, 0, 2)).reshape(C, C)  # [c, h*64]
        w4 = w2.reshape(N_CC, 128, N_HP, 128).transpose(1, 2, 0, 3)   # [128c, hp, cc, 128]
        return (qk_scale *
                np.ascontiguousarray(w4).reshape(128, -1)).astype(qk_np)

    wq_h = pack_qk(Wq)
    wk_h = pack_qk(Wk)
    v8 = cfg.get("vproj", "") == "fp8"
    v_np = mybir.dt.np(F8E4) if v8 else proj_np
    v_scale = W8_SCALE if v8 else 1.0
    wv2 = np.ascontiguousarray(Wv.transpose(1, 0, 2)).reshape(C, C)    # [c, hd]
    wv_h = (v_scale * np.ascontiguousarray(
        wv2.reshape(N_CC, 128, C).transpose(1, 0, 2)).reshape(128, -1)).astype(v_np)
    wo_h = np.ascontiguousarray(
        Wo.reshape(N_CC, 128, C).transpose(1, 0, 2)).reshape(128, -1).astype(op_np)

    ii, jj = np.indices((128, 128))
    tri = (jj >= ii).astype(np.float32)     # [s, t]: keep t >= s
    mask_h = np.concatenate(
        [tri, np.ones((128, 128), np.float32), tri], axis=1).astype(pv_np)
    sc_np = np_dt(cfg["scores"])
    qk8 = cfg.get("qkproj", "") == "fp8"
    negm_h = np.where(jj >= ii, 0.0, -60000.0).astype(sc_np)
    ident_h = np.eye(128, dtype=np.float32).astype(op_np)
    # qk8: scores are W8_SCALE^2 larger, the exp scale W8_SCALE^2 smaller;
    # boost the mask-add by W8_SCALE so it still flushes exp to zero
    identm_h = ((W8_SCALE if qk8 else 1.0) *
                np.eye(128, dtype=np.float32)).astype(sc_np)

    in_maps = []
    for core in range(N_CORES):
        xs = x[core * B_LOC:(core + 1) * B_LOC]              # [8, 256, 512]
        xTf = np.ascontiguousarray(xs.transpose(0, 2, 1))
        m = {
            "xT": xTf.astype(proj_np), "wq": wq_h, "wk": wk_h, "wv": wv_h,
            "wo": wo_h, "mask": mask_h, "negm": negm_h, "ident": ident_h,
            "identm": identm_h,
        }
        if qk8 or v8:
            m["xT8"] = xTf.astype(mybir.dt.np(F8E4))
        in_maps.append(m)
    return in_maps


DEFAULT_CFG = {"proj": "fp16", "scores": "fp16", "pv": "fp16",
               "outproj": "fp16", "qkproj": "fp16", "vproj": "fp16"}
DEFAULT_TUNE = {}

_NC_CACHE = {}


def run(x, Wq, Wk, Wv, Wo, cfg=None, trace=False):
    cfg = cfg or DEFAULT_CFG
    key = (tuple(sorted(cfg.items())), tuple(sorted(DEFAULT_TUNE.items())))
    if key not in _NC_CACHE:
        _NC_CACHE[key] = build_nc(cfg)
    nc = _NC_CACHE[key]
    in_maps = _prep_inputs(np.asarray(x), np.asarray(Wq), np.asarray(Wk),
                           np.asarray(Wv), np.asarray(Wo), cfg)
    res = run_bass_kernel_spmd(nc, in_maps, core_ids=list(range(N_CORES)),
                               trace=trace)
    y = np.concatenate([r["y"] for r in res.results], axis=0)
    return y, res


def kernel(x, Wq, Wk, Wv, Wo):
    y, _ = run(x, Wq, Wk, Wv, Wo)
    return y.astype(np.float32)


if __name__ == "__main__":
    import time
    t0 = time.time()
    nc = build_nc(DEFAULT_CFG)
    print(f"build+compile: {time.time()-t0:.1f}s")
